# revision 14
# baseline (speedup 1.0000x reference)
"""Trainium2 Bass kernel for nn_CharacterModel (char-LSTM + masked sigmoid attention).

Strategy (v2):
  - Data-parallel over words: core c gets sorted words c::8 (lengths stay sorted
    descending per core), 1024 words/core. Per-step active counts nts[t] are the
    global schedule (ceil/8, padded to >=256); boundary/padded words are
    neutralized by a -60 attention logit (sigmoid -> 0).
  - Layout: hidden/gate dims on partitions, words on the free axis; hid = 128*j+p.
  - fp8-e4m3 DoubleRow for the recurrent W_hh@h matmul: h is stored fp8 in
    [128, 2(j), slot] form, so the two K=128 passes become one DR pass.
    W_ih@x stays bf16 (K=65 incl. a ones-row carrying the bias; at t=0 the
    bias row also folds in W_hh@h_init, skipping the recurrent matmul).
  - Gate PSUM is [i,f,o][g] so ONE activation does all three sigmoids.
  - Attention is deferred into per-slot h stores (fp8 for the dot matmul, bf16
    for the weighted pool) and processed right after each step, overlapping the
    next step's matmuls. res is accumulated f32, split per-j across DVE/GPSIMD.
  - A short warmup matmul burst at kernel start flips the HAM clock gate to 8/8
    before the real recurrence begins.
  - Host folds the unsort permutation + reshape into the unshard.
"""

import math
import os

import numpy as np
import ml_dtypes

N_WORDS = 8192
MAX_WLEN = 16
EMB = 64
HID = 256
N_SENT = 256
SENT_LEN = 32
NCORES = 8
W = N_WORDS // NCORES  # 1024 words per core
CHUNK = 512

BF16 = ml_dtypes.bfloat16
FP8 = ml_dtypes.float8_e4m3

_BUILD_CACHE = {}
last_result = None  # stashes the most recent BassKernelResults (for profiling)

# column permutation: new col (Gi*2+j)*128+m  <-  torch row base + 128j + m,
# G order [i, f, o, g] -> torch bases [0, 256, 768, 512]
_PERM = np.empty(4 * HID, np.int64)
for _gi, _base in enumerate([0, 256, 768, 512]):
    for _j in range(2):
        _PERM[(_gi * 2 + _j) * 128: (_gi * 2 + _j + 1) * 128] = \
            _base + 128 * _j + np.arange(128)

N_WARMUP = 10

def _chunks(n):
    if n > CHUNK:
        return [(0, CHUNK), (CHUNK, n)]
    return [(0, n)]


def _build(nts):
    """Build + schedule the Bass program for a given per-step word-count schedule."""
    import concourse.tile as tile
    import concourse.mybir as mybir
    from concourse import bacc

    f32 = mybir.dt.float32
    bf16 = mybir.dt.bfloat16
    fp8 = mybir.dt.float8e4
    AF = mybir.ActivationFunctionType
    OP = mybir.AluOpType
    DR = mybir.MatmulPerfMode.DoubleRow

    offs = [0]
    for t in range(MAX_WLEN):
        offs.append(offs[-1] + nts[t])
    TOT = offs[MAX_WLEN]
    TOTP = (TOT + 15) // 16 * 16  # DR moving k-pair step must be %16


    nc = bacc.Bacc("TRN2", name="char_lstm2")

    d_embs = nc.dram_tensor("embs", [MAX_WLEN, EMB + 1, W], bf16, kind="ExternalInput")
    d_wih0 = nc.dram_tensor("wih0", [EMB + 1, 4 * HID], bf16, kind="ExternalInput")
    d_wih = nc.dram_tensor("wih", [EMB + 1, 4 * HID], bf16, kind="ExternalInput")
    d_whh = nc.dram_tensor("whh", [128, 2, 4 * HID], fp8, kind="ExternalInput")
    d_attn = nc.dram_tensor("attn", [128, 2], bf16, kind="ExternalInput")
    d_mask = nc.dram_tensor("masklog", [1, TOT], bf16, kind="ExternalInput")
    d_c0 = nc.dram_tensor("c0", [128, 2], f32, kind="ExternalInput")
    d_out = nc.dram_tensor("res", [128, 2, W], f32, kind="ExternalOutput")

    with tile.TileContext(nc) as tc:
        with (
            tc.tile_pool(name="const", bufs=1) as cp,
            tc.tile_pool(name="embp", bufs=3) as ep,
            tc.tile_pool(name="sigp", bufs=3) as gp,
            tc.tile_pool(name="workp", bufs=3) as wp,
            tc.tile_pool(name="state", bufs=1) as sp,
            tc.tile_pool(name="psif", bufs=2, space="PSUM") as pg,
            tc.tile_pool(name="pgx", bufs=2, space="PSUM") as px,
        ):
            # --- constants (critical-path DMAs on sync; bulk on gpsimd queue) ---
            wih0 = cp.tile([EMB + 1, 4 * HID], bf16, tag="wih0")
            nc.sync.dma_start(wih0[:], d_wih0[:])
            wih = cp.tile([EMB + 1, 4 * HID], bf16, tag="wih")
            nc.gpsimd.dma_start(wih[:], d_wih[:])
            whh = cp.tile([128, 2, 4 * HID], fp8, tag="whh")
            nc.gpsimd.dma_start(whh[:], d_whh[:])
            attn = cp.tile([128, 2], bf16, tag="attn")
            nc.gpsimd.dma_start(attn[:], d_attn[:])
            maskr = cp.tile([1, TOT], bf16, tag="maskr")
            nc.gpsimd.dma_start(maskr[:], d_mask[:])
            c0t = cp.tile([128, 2], f32, tag="c0")
            nc.gpsimd.dma_start(c0t[:], d_c0[:])
            ones128 = cp.tile([1, 128], bf16, tag="ones128")
            nc.vector.memset(ones128[:], 1.0)
            one1 = cp.tile([1, 1], bf16, tag="one1")
            nc.vector.memset(one1[:], 1.0)
            wmv = cp.tile([1, CHUNK], bf16, tag="wmv")
            nc.vector.memset(wmv[:], 0.0)

            # --- state ---
            hst = sp.tile([128, 2, TOTP], fp8, tag="hst")
            hbf0 = sp.tile([128, TOT], bf16, tag="hbf0")
            hbf1 = sp.tile([128, TOT], bf16, tag="hbf1")
            cst0 = sp.tile([128, W], bf16, tag="cst0")
            cst1 = sp.tile([128, W], bf16, tag="cst1")
            res0 = sp.tile([128, W], f32, tag="res0")
            res1 = sp.tile([128, W], f32, tag="res1")
            cst = [cst0, cst1]
            res = [res0, res1]

            # --- HAM warmup: dense PE burst while the first DMAs land ---
            wps = px.tile([128, CHUNK], f32, tag="gx")
            for _ in range(N_WARMUP):
                nc.tensor.matmul(wps[:, :], ones128[:, :], wmv[:1, :],
                                 start=True, stop=True)

            hbf = [hbf0, hbf1]
            for t in range(MAX_WLEN):
                n = nts[t]
                if n == 0:
                    break
                off = offs[t]
                poff = offs[t - 1] if t > 0 else 0
                wih_t = wih0 if t == 0 else wih

                embt = ep.tile([EMB + 1, W], bf16, tag="embt")
                if t == 0:
                    nc.sync.dma_start(embt[:, :CHUNK], d_embs[t, :, :CHUNK])
                    nc.scalar.dma_start(embt[:, CHUNK:n], d_embs[t, :, CHUNK:n])
                else:
                    nc.sync.dma_start(embt[:, :n], d_embs[t, :, :n])

                for (w0, we) in _chunks(n):
                    cw = we - w0
                    for j in range(2):
                        sifp = pg.tile([128, 3, CHUNK], f32, tag="sif")
                        gps = px.tile([128, CHUNK], f32, tag="gx")
                        for gi in range(3):  # i, f, o
                            col = (gi * 2 + j) * 128
                            if t > 0:
                                nc.tensor.matmul(
                                    sifp[:, gi, :cw], whh[:, :, col:col + 128],
                                    hst[:, :, poff + w0:poff + we],
                                    start=True, stop=False, perf_mode=DR)
                                nc.tensor.matmul(
                                    sifp[:, gi, :cw], wih_t[:, col:col + 128],
                                    embt[:, w0:we], start=False, stop=True)
                            else:
                                nc.tensor.matmul(
                                    sifp[:, gi, :cw], wih_t[:, col:col + 128],
                                    embt[:, w0:we], start=True, stop=True)
                        col = (3 * 2 + j) * 128  # g gate
                        if t > 0:
                            nc.tensor.matmul(
                                gps[:, :cw], whh[:, :, col:col + 128],
                                hst[:, :, poff + w0:poff + we],
                                start=True, stop=False, perf_mode=DR)
                            nc.tensor.matmul(
                                gps[:, :cw], wih_t[:, col:col + 128],
                                embt[:, w0:we], start=False, stop=True)
                        else:
                            nc.tensor.matmul(
                                gps[:, :cw], wih_t[:, col:col + 128],
                                embt[:, w0:we], start=True, stop=True)

                        # --- activations ---
                        sig = gp.tile([128, 3, CHUNK], bf16, tag="sig")
                        nc.scalar.activation(sig[:, :, :cw], sifp[:, :, :cw], AF.Sigmoid)
                        tg = wp.tile([128, CHUNK], bf16, tag="tg")
                        nc.scalar.activation(tg[:, :cw], gps[:, :cw], AF.Tanh)

                        # --- cell update (DVE + GPSIMD) ---
                        ig = wp.tile([128, CHUNK], bf16, tag="ig")
                        nc.vector.tensor_tensor(ig[:, :cw], sig[:, 0, :cw], tg[:, :cw], OP.mult)
                        if t == 0:
                            nc.vector.scalar_tensor_tensor(
                                cst[j][:, w0:we], sig[:, 1, :cw], c0t[:, j:j + 1],
                                ig[:, :cw], OP.mult, OP.add)
                        else:
                            cm = wp.tile([128, CHUNK], bf16, tag="cm")
                            nc.vector.tensor_tensor(cm[:, :cw], sig[:, 1, :cw], cst[j][:, w0:we], OP.mult)
                            nc.gpsimd.tensor_tensor(cst[j][:, w0:we], cm[:, :cw], ig[:, :cw], OP.add)
                        tc_ = wp.tile([128, CHUNK], bf16, tag="tc")
                        nc.scalar.activation(tc_[:, :cw], cst[j][:, w0:we], AF.Tanh)
                        nc.vector.tensor_tensor(
                            hbf[j][:, off + w0:off + we], sig[:, 2, :cw], tc_[:, :cw], OP.mult)
                        nc.gpsimd.tensor_tensor(
                            hst[:, j, off + w0:off + we], sig[:, 2, :cw], tc_[:, :cw], OP.mult)

                # --- deferred attention for slot t (overlaps step t+1 matmuls) ---
                for (w0, we) in _chunks(n):
                    cw = we - w0
                    ap_ = px.tile([128, CHUNK], f32, tag="gx")
                    nc.tensor.matmul(ap_[:1, :cw], attn[:, 0:1],
                                     hbf0[:, off + w0:off + we],
                                     start=True, stop=False)
                    nc.tensor.matmul(ap_[:1, :cw], attn[:, 1:2],
                                     hbf1[:, off + w0:off + we],
                                     start=False, stop=False)
                    nc.tensor.matmul(ap_[:1, :cw], one1[:1, :1],
                                     maskr[:1, off + w0:off + we],
                                     start=False, stop=True)
                    wr = wp.tile([1, CHUNK], bf16, tag="wr")
                    nc.scalar.activation(wr[:1, :cw], ap_[:1, :cw], AF.Sigmoid)
                    nc.tensor.matmul(ap_[:, :cw], ones128[:1, :], wr[:1, :cw],
                                     start=True, stop=True)
                    if t == 0:
                        # GPSIMD cannot read PSUM -> both res writes on DVE
                        nc.vector.tensor_tensor(
                            res[0][:, w0:we], hbf0[:, off + w0:off + we], ap_[:, :cw], OP.mult)
                        nc.vector.tensor_tensor(
                            res[1][:, w0:we], hbf1[:, off + w0:off + we], ap_[:, :cw], OP.mult)
                    else:
                        wh0 = wp.tile([128, CHUNK], bf16, tag="wh0")
                        nc.vector.tensor_tensor(
                            wh0[:, :cw], hbf0[:, off + w0:off + we], ap_[:, :cw], OP.mult)
                        wh1 = wp.tile([128, CHUNK], bf16, tag="wh1")
                        nc.vector.tensor_tensor(
                            wh1[:, :cw], hbf1[:, off + w0:off + we], ap_[:, :cw], OP.mult)
                        nc.gpsimd.tensor_tensor(
                            res[0][:, w0:we], res[0][:, w0:we], wh0[:, :cw], OP.add)
                        nc.gpsimd.tensor_tensor(
                            res[1][:, w0:we], res[1][:, w0:we], wh1[:, :cw], OP.add)

                # words [n_{t+1}, n_t) retire after this slot's attention
                n_next = nts[t + 1] if t + 1 < MAX_WLEN else 0
                if n_next < n:
                    nc.sync.dma_start(d_out[:, 0, n_next:n], res[0][:, n_next:n])
                    nc.sync.dma_start(d_out[:, 1, n_next:n], res[1][:, n_next:n])

            if nts[MAX_WLEN - 1] > 0:
                nf = nts[MAX_WLEN - 1]
                nc.sync.dma_start(d_out[:, 0, :nf], res[0][:, :nf])
                nc.sync.dma_start(d_out[:, 1, :nf], res[1][:, :nf])

    nc.compile()
    return nc


def _get_nc(nts):
    key = tuple(nts)
    if key not in _BUILD_CACHE:
        _BUILD_CACHE[key] = _build(key)
    return _BUILD_CACHE[key]


def _prep_inputs(chars, wordlens, emb_table, W_ih, W_hh, b_ih, b_hh,
                 attn_w, h_init, c_init, nts):
    offs = np.concatenate([[0], np.cumsum(nts)]).astype(np.int64)
    TOT = int(offs[MAX_WLEN])
    bias = b_ih + b_hh
    bias0 = bias + W_hh @ h_init
    wihT = np.concatenate([W_ih.T, bias[None, :]], axis=0)[:, _PERM]
    wih0T = np.concatenate([W_ih.T, bias0[None, :]], axis=0)[:, _PERM]
    whh_dr = W_hh.T[:, _PERM].reshape(2, 128, 4 * HID).transpose(1, 0, 2)
    shared = {
        "wih": wihT.astype(BF16),
        "wih0": wih0T.astype(BF16),
        "whh": np.ascontiguousarray(whh_dr).astype(FP8),
        "attn": np.ascontiguousarray(attn_w.reshape(2, 128).T).astype(BF16),
        "c0": np.ascontiguousarray(c_init.reshape(2, 128).T).astype(np.float32),
    }
    steps = np.arange(MAX_WLEN)[:, None]
    in_maps = []
    for cid in range(NCORES):
        idx = np.arange(W) * NCORES + cid
        embs = emb_table[chars[idx]]            # [W, 16, 64]
        embsT = np.ones((MAX_WLEN, EMB + 1, W), np.float32)
        embsT[:, :EMB, :] = embs.transpose(1, 2, 0)
        lens = wordlens[idx]
        mfull = np.where(lens[None, :] > steps, 0.0, -60.0).astype(np.float32)  # [16, W]
        mpacked = np.zeros((1, TOT), np.float32)
        for t in range(MAX_WLEN):
            if nts[t]:
                mpacked[0, offs[t]:offs[t + 1]] = mfull[t, :nts[t]]
        in_maps.append({
            **shared,
            "embs": embsT.astype(BF16),
            "masklog": mpacked.astype(BF16),
        })
    return in_maps


def kernel(chars, wordlens, word_orig_idx, emb_table, W_ih, W_hh, b_ih, b_hh,
           attn_w, h_init, c_init):
    global last_result
    from concourse.bass_utils import run_bass_kernel_spmd

    chars = np.asarray(chars)
    wordlens = np.asarray(wordlens)
    word_orig_idx = np.asarray(word_orig_idx)
    emb_table = np.asarray(emb_table, dtype=np.float32)
    W_ih = np.asarray(W_ih, dtype=np.float32)
    W_hh = np.asarray(W_hh, dtype=np.float32)
    b_ih = np.asarray(b_ih, dtype=np.float32)
    b_hh = np.asarray(b_hh, dtype=np.float32)
    attn_w = np.asarray(attn_w, dtype=np.float32)
    h_init = np.asarray(h_init, dtype=np.float32)
    c_init = np.asarray(c_init, dtype=np.float32)

    # per-step active word counts (identical schedule on every core); tail steps
    # padded to 256 (mask neutralizes the extras; keeps chunks big + HAM warm)
    nts = tuple(int(math.ceil(int((wordlens > t).sum()) / NCORES)) for t in range(MAX_WLEN))
    nts = tuple(max(v, 256) if v > 0 else 0 for v in nts)
    nc = _get_nc(nts)

    in_maps = _prep_inputs(chars, wordlens, emb_table, W_ih, W_hh, b_ih, b_hh,
                           attn_w, h_init, c_init, nts)

    last_result = run_bass_kernel_spmd(
        nc, in_maps, core_ids=list(range(NCORES)),
        trace=bool(int(os.environ.get("KERNEL_TRACE", "0"))),
    )

    res_sorted = np.zeros((N_WORDS, HID), np.float32)
    for cid in range(NCORES):
        rc = np.asarray(last_result.results[cid]["res"])  # [128, 2, W]
        res_sorted[np.arange(W) * NCORES + cid] = rc.transpose(2, 1, 0).reshape(W, HID)

    out = np.zeros_like(res_sorted)
    out[word_orig_idx] = res_sorted
    return out.reshape(N_SENT, SENT_LEN, HID)
